# revision 1
# baseline (speedup 1.0000x reference)
"""CSWinBlock3D Trainium2 kernel (8-core SPMD, data-parallel over depth).

Layout: channels-major [C, T] (matches x's DRAM layout [1, C, D, H, W]).
Each core handles 4 depth slices = 4096 tokens. No collectives.
"""

import sys

sys.path.insert(0, "/opt/trn_rl_repo")

from contextlib import ExitStack

import numpy as np

import concourse.bass as bass
import concourse.bacc as bacc
import concourse.tile as tile
from concourse import mybir

F32 = mybir.dt.float32
F32R = mybir.dt.float32r
BF16 = mybir.dt.bfloat16
AF = mybir.ActivationFunctionType
ALU = mybir.AluOpType

N_CORES = 8
C = 512
RESO = 32
SPLIT = 4
HH = 8          # heads per branch
HD = 32         # head dim
CB = 256        # channels per branch
HID = 2048
EPS = 1e-5
SCALE = HD ** -0.5
NSLICE = 4      # depth slices per core
TOK = 1024      # tokens per depth slice
TCORE = NSLICE * TOK  # 4096 tokens per core
NCH = C // 128  # 4 channel chunks
NHC = HID // 128  # 16 hidden chunks


def bc(ap):
    return ap.bitcast(F32R)


def build_kernel(gelu_func=AF.Gelu, stage=5, loops=1, loops_a=None, loops_b=None):
    nc = bacc.Bacc("TRN2", target_bir_lowering=False, debug=False,
                   num_devices=N_CORES)

    dram = {}
    def din(name, shape):
        dram[name] = nc.dram_tensor(name, list(shape), F32, kind="ExternalInput").ap()
    din("x", (C, TCORE))
    din("norm1_g", (C,)); din("norm1_b", (C,))
    din("qkv_w", (C, 3 * C))
    din("lepe0_w", (CB, 9)); din("lepe0_b", (CB,))
    din("lepe1_w", (CB, 9)); din("lepe1_b", (CB,))
    din("proj_w", (C, C)); din("proj_b", (C,))
    din("norm2_g", (C,)); din("norm2_b", (C,))
    din("fc1_w", (C, HID)); din("fc1_b", (HID,))
    din("fc2_w", (HID, C)); din("fc2_b", (C,))
    out_d = nc.dram_tensor("out", [C, TCORE], F32, kind="ExternalOutput").ap()
    xf_d = nc.dram_tensor("xf_scratch", [C, TCORE], F32, kind="Internal").ap()

    import ml_dtypes
    ident_d = nc.inline_tensor(np.eye(128, dtype=np.float32), name="ident128")
    ones128_d = nc.inline_tensor(np.ones((128, 128), dtype=np.float32), name="ones128c")
    ones1_d = nc.inline_tensor(np.ones((1, 512), dtype=np.float32), name="ones1c")
    zeros_d = nc.inline_tensor(
        np.zeros((128, 8 * 204), dtype=ml_dtypes.bfloat16), name="zerosc")

    with ExitStack() as ctx:
        tc = ctx.enter_context(tile.TileContext(nc))
        csts = ctx.enter_context(tc.tile_pool(name="csts", bufs=1))

        # ---- constants ----
        ones128 = csts.tile([128, 128], F32, tag="ones128", name="ones128")
        nc.sync.dma_start(out=bc(ones128), in_=bc(ones128_d.ap()))
        ones1 = csts.tile([1, 512], F32, tag="ones1", name="ones1")
        nc.sync.dma_start(out=bc(ones1), in_=bc(ones1_d.ap()))
        eps_t = csts.tile([128, 1], F32, tag="eps_t", name="eps_t")
        nc.gpsimd.memset(eps_t, EPS)
        zero_t = csts.tile([128, 1], F32, tag="zero_t", name="zero_t")
        nc.gpsimd.memset(zero_t, 0.0)

        def load_pcol(name, nchunk):
            # [nchunk*128] dram -> [128, nchunk] sbuf (col c = chunk c)
            t = csts.tile([128, nchunk], F32, tag=name, name=name)
            nc.sync.dma_start(out=t, in_=dram[name].rearrange("(c p) -> p c", p=128))
            return t
        g1t = load_pcol("norm1_g", NCH); b1t = load_pcol("norm1_b", NCH)
        g2t = load_pcol("norm2_g", NCH); b2t = load_pcol("norm2_b", NCH)
        fc1b = load_pcol("fc1_b", NHC)

        def load_row(name):
            t = csts.tile([1, 512], F32, tag=name, name=name)
            nc.sync.dma_start(out=bc(t), in_=bc(dram[name].rearrange("(a f) -> a f", a=1)))
            return t
        pb = load_row("proj_b"); fc2b = load_row("fc2_b")

        lb = []
        lw = []
        for br in range(2):
            lwn = f"lepe{br}_w"
            lwt = []
            for ch in range(2):
                t = csts.tile([128, 9], F32, tag=f"{lwn}_{ch}", name=f"{lwn}_{ch}")
                nc.sync.dma_start(out=t, in_=dram[lwn][128 * ch:128 * (ch + 1), :])
                lwt.append(t)
            lw.append(lwt)
            lbn = f"lepe{br}_b"
            t = csts.tile([128, 2], F32, tag=lbn, name=lbn)
            nc.sync.dma_start(out=t, in_=dram[lbn].rearrange("(c p) -> p c", p=128))
            lb.append(t)

        # =============== helpers ===============
        def ln_stats(src_ap, pools):
            """LayerNorm stats for one 512-token group -> (negm, rb)."""
            psq, pstat, ps = pools
            xsq = []
            for ch in range(NCH):
                t = psq.tile([128, 512], F32, tag="xsq", name="xsq")
                nc.scalar.activation(bc(t), src_ap(ch), AF.Square, bias=zero_t)
                xsq.append(t)
            sb = ps.tile([128, 512], F32, tag="mm", name="mm")
            for k in range(NCH):
                nc.tensor.matmul(sb, bc(ones128), bc(src_ap(k)),
                                 start=(k == 0), stop=(k == NCH - 1))
            qb = ps.tile([128, 512], F32, tag="mm", name="mm")
            for k in range(NCH):
                nc.tensor.matmul(qb, bc(ones128), bc(xsq[k]),
                                 start=(k == 0), stop=(k == NCH - 1))
            negm = pstat.tile([128, 512], F32, tag="negm", name="negm", bufs=2)
            nc.vector.tensor_scalar_mul(negm, sb, -1.0 / C)
            tq = pstat.tile([128, 512], F32, tag="tq", name="tq")
            nc.vector.tensor_scalar_mul(tq, qb, 1.0 / C)
            m2 = pstat.tile([128, 512], F32, tag="m2", name="m2")
            nc.vector.tensor_mul(m2, negm, negm)
            var = pstat.tile([128, 512], F32, tag="var", name="var")
            nc.vector.tensor_sub(var, tq, m2)
            sd = pstat.tile([128, 512], F32, tag="sd", name="sd")
            nc.scalar.activation(sd, var, AF.Ln, bias=eps_t)
            rb = pstat.tile([128, 512], F32, tag="rb", name="rb", bufs=2)
            nc.scalar.activation(rb, sd, AF.Exp, bias=zero_t, scale=-0.5)
            return negm, rb

        def ln_apply(src_ap, dst_ap, negm, rb, g_sb, b_sb, pstat):
            for ch in range(NCH):
                u = pstat.tile([128, 512], F32, tag="u", name="u")
                nc.gpsimd.tensor_add(u, src_ap(ch), negm)
                v1 = pstat.tile([128, 512], F32, tag="v1", name="v1")
                nc.vector.tensor_mul(v1, u, rb)
                nc.vector.tensor_scalar(bc(dst_ap(ch)), v1,
                                        g_sb[:, ch:ch + 1], b_sb[:, ch:ch + 1],
                                        op0=ALU.mult, op1=ALU.add)

        def ln_group(src_ap, dst_ap, g_sb, b_sb, pools):
            negm, rb = ln_stats(src_ap, pools)
            ln_apply(src_ap, dst_ap, negm, rb, g_sb, b_sb, pools[1])

        # =============== PHASE A ===============
        with ExitStack() as actx:
            wA = actx.enter_context(tc.tile_pool(name="wA", bufs=1))
            ident = wA.tile([128, 128], F32, tag="ident", name="ident")
            nc.sync.dma_start(out=ident, in_=ident_d.ap())
            # diag matrices for lepe: dgb[br][ch][tap] = diag(w[128ch.., tap])
            dgb = [[[None] * 9 for _ in range(2)] for _ in range(2)]
            for br in range(2):
                for ch in range(2):
                    for tap in range(9):
                        t = wA.tile([128, 128], BF16, tag=f"dgb{br}{ch}{tap}",
                                    name=f"dgb{br}{ch}{tap}")
                        nc.vector.tensor_scalar_mul(t, ident,
                                                    lw[br][ch][:, tap:tap + 1])
                        dgb[br][ch][tap] = t
            qkvw = []
            for k in range(NCH):
                t = wA.tile([128, 3 * C], F32, tag=f"qkvw{k}", name=f"qkvw{k}")
                nc.sync.dma_start(out=bc(t), in_=bc(dram["qkv_w"][128 * k:128 * (k + 1), :]))
                qkvw.append(t)
            projw = []
            for k in range(NCH):
                t = wA.tile([128, C], F32, tag=f"projw{k}", name=f"projw{k}")
                nc.sync.dma_start(out=bc(t), in_=bc(dram["proj_w"][128 * k:128 * (k + 1), :]))
                projw.append(t)
            px = actx.enter_context(tc.tile_pool(name="px", bufs=4))
            pimg = actx.enter_context(tc.tile_pool(name="pimg", bufs=4))
            pattT = actx.enter_context(tc.tile_pool(name="pattT", bufs=4))
            pqkv = actx.enter_context(tc.tile_pool(name="pqkv", bufs=1))
            psq = actx.enter_context(tc.tile_pool(name="psq", bufs=2))
            pstat = actx.enter_context(tc.tile_pool(name="pstat", bufs=1))
            pw = actx.enter_context(tc.tile_pool(name="pw", bufs=3))
            pvtm = actx.enter_context(tc.tile_pool(name="pvtm", bufs=4))
            pxfo = actx.enter_context(tc.tile_pool(name="pxfo", bufs=2))
            pvpad = actx.enter_context(tc.tile_pool(name="pvpad", bufs=1))
            # zero-halo V buffers: per (branch, chunk), halo zeroed once
            vpad = [[pvpad.tile([128, 8 * 204], BF16, tag=f"vpad{b}{ch}",
                                name=f"vpad{b}{ch}") for ch in range(2)]
                    for b in range(2)]
            for b in range(2):
                for ch in range(2):
                    nc.sync.dma_start(out=vpad[b][ch], in_=zeros_d.ap())
            ps_mm = actx.enter_context(tc.tile_pool(name="ps_mm", bufs=2, space="PSUM"))
            ps_ot = actx.enter_context(tc.tile_pool(name="ps_ot", bufs=2, space="PSUM"))
            ps_sm = actx.enter_context(tc.tile_pool(name="ps_sm", bufs=2, space="PSUM"))

            import contextlib
            la = loops_a if loops_a is not None else loops
            loopA = tc.For_i(0, la, 1) if la > 1 else contextlib.nullcontext()
            with loopA:
              for sl in range(NSLICE if stage >= 4 else 1):
                # load x slice (channels-major, raw token order)
                xs = []
                for ch in range(NCH):
                    t = px.tile([128, TOK], F32, tag="x", name="x")
                    nc.sync.dma_start(
                        out=bc(t), in_=bc(dram["x"][128 * ch:128 * (ch + 1),
                                                    TOK * sl:TOK * (sl + 1)]))
                    xs.append(t)

                # LN1 -> img
                img = [pimg.tile([128, TOK], F32, tag="img", name="img") for _ in range(NCH)]
                for g2 in range(2):
                    ln_group(lambda ch: xs[ch][:, 512 * g2:512 * (g2 + 1)],
                             lambda ch: img[ch][:, 512 * g2:512 * (g2 + 1)],
                             g1t, b1t, (psq, pstat, ps_mm))

                if stage == 1:
                    for ch in range(NCH):
                        nc.sync.dma_start(
                            out=out_d[128 * ch:128 * (ch + 1), 0:TOK], in_=img[ch])
                    continue
                attT = [pattT.tile([128, TOK], F32, tag="attT", name="attT") for _ in range(NCH)]

                for br in range(2):
                    # ---- qkv for this branch (window-ordered for br 0) ----
                    # q,k: head-folded [32, 4 heads x 1024 tok] bf16 (QK matmuls
                    # need lhsT/rhs at partition base 0 - row tiling faults on hw)
                    qkf = {}
                    vb = []
                    for m in range(3):  # q, k, v
                        for G in range(2):
                            if m < 2:
                                tb = pqkv.tile([128, TOK], BF16, tag=f"qkb{m}{G}",
                                               name=f"qkb{m}{G}")
                                t = pqkv.tile([32, 4 * TOK], BF16,
                                              tag=f"qkf{m}{G}", name=f"qkf{m}{G}")
                            else:
                                t = pqkv.tile([128, TOK], F32, tag=f"qkv{m}{G}",
                                              name=f"qkv{m}{G}")
                            oc = 4 * m + 2 * br + G
                            for g2 in range(2):
                                pp = ps_mm.tile([128, 512], F32, tag="mm", name="mm")
                                for k in range(NCH):
                                    if br == 0:
                                        rhs = img[k].rearrange(
                                            "p (h j w) -> p j h w", h=32, j=8, w=4
                                        )[:, 4 * g2:4 * (g2 + 1), :, :]
                                    else:
                                        rhs = img[k][:, 512 * g2:512 * (g2 + 1)]
                                    nc.tensor.matmul(
                                        pp, bc(qkvw[k][:, 128 * oc:128 * (oc + 1)]),
                                        bc(rhs), start=(k == 0), stop=(k == NCH - 1))
                                if m < 2:
                                    nc.scalar.copy(tb[:, 512 * g2:512 * (g2 + 1)], pp)
                                else:
                                    nc.scalar.copy(bc(t[:, 512 * g2:512 * (g2 + 1)]), pp)
                            if m < 2:
                                for i in range(4):
                                    nc.sync.dma_start(
                                        out=t[0:32, 1024 * i:1024 * (i + 1)],
                                        in_=tb[32 * i:32 * (i + 1), :])
                                qkf[(m, G)] = t
                            else:
                                vb.append(t)
                    qf = [qkf[(0, 0)], qkf[(0, 1)]]
                    kf = [qkf[(1, 0)], qkf[(1, 1)]]
                    if stage == 2:
                        if br == 0:
                            for ch in range(4):
                                nc.sync.dma_start(
                                    out=out_d[128 * ch:128 * (ch + 1), 0:TOK],
                                    in_=[qb[0], qb[1], kb[0], vb[1]][ch])
                        continue

                    # ---- attention ----
                    Y, X = (32, 4) if br == 0 else (4, 32)
                    # fill zero-halo V interiors for lepe
                    for ch2 in range(2):
                        for win in range(8):
                            nc.vector.tensor_copy(
                                vpad[br][ch2].rearrange(
                                    "p (s y x) -> p s y x", s=8, y=Y + 2, x=X + 2
                                )[:, win, 1:Y + 1, 1:X + 1],
                                vb[ch2].rearrange(
                                    "p (s y x) -> p s y x", s=8, y=Y, x=X)[:, win])
                    for half in range(2):
                        # V tokens-major for the 4 windows of this half
                        vtm = []
                        for wl in range(4):
                            win = 4 * half + wl
                            tp = ps_sm.tile([128, 256], F32, tag="sm", name="sm")
                            for ch2 in range(2):
                                nc.tensor.transpose(
                                    tp[:, 128 * ch2:128 * (ch2 + 1)],
                                    vb[ch2][:, 128 * win:128 * (win + 1)],
                                    ident)
                            vt = pvtm.tile([128, 256], F32, tag="vtm", name="vtm")
                            nc.vector.tensor_copy(bc(vt), tp)
                            vtm.append(vt)
                        for G in range(2):
                            otb = ps_ot.tile([128, 512], F32, tag="ot", name="ot")
                            # lepe depthwise taps (center first: start=True)
                            taps = [(1, 1)] + [(dy, dx) for dy in range(3)
                                               for dx in range(3) if (dy, dx) != (1, 1)]
                            for (dy, dx) in taps:
                                srcap = vpad[br][G].rearrange(
                                    "p (s y x) -> p s y x", s=8, y=Y + 2, x=X + 2
                                )[:, 4 * half:4 * (half + 1),
                                  dy:dy + Y, dx:dx + X]
                                nc.tensor.matmul(
                                    otb, dgb[br][G][3 * dy + dx],
                                    srcap, start=(dy == 1 and dx == 1),
                                    stop=False, skip_group_check=True)
                            def emit_front(wl):
                                win = 4 * half + wl
                                sx = ps_sm.tile([128, 512], F32, tag="sm", name="sm")
                                for i in range(4):
                                    nc.tensor.matmul(
                                        sx[:, 128 * i:128 * (i + 1)],
                                        kf[G][0:32, 1024 * i + 128 * win:
                                              1024 * i + 128 * (win + 1)],
                                        qf[G][0:32, 1024 * i + 128 * win:
                                              1024 * i + 128 * (win + 1)],
                                        start=True, stop=True,
                                        skip_group_check=True)
                                pt = pw.tile([128, 512], F32, tag="pt", name="pt")
                                nc.scalar.activation(bc(pt), sx, AF.Exp, bias=zero_t,
                                                     scale=SCALE)
                                return pt

                            def emit_back(wl, pt):
                                sv = ps_sm.tile([128, 8], F32, tag="sv", name="sv", bufs=1)
                                for i in range(4):
                                    nc.tensor.matmul(
                                        sv[:, 2 * i:2 * i + 2],
                                        bc(pt[:, 128 * i:128 * (i + 1)]),
                                        bc(ones128[:, 0:2]),
                                        start=True, stop=True,
                                        skip_group_check=True)
                                rv = pw.tile([128, 4], F32, tag="rv", name="rv")
                                nc.vector.reciprocal(rv, sv.rearrange(
                                    "p (a b) -> p a b", a=4, b=2)[:, :, 0])
                                ou = ps_sm.tile([128, 128], F32, tag="ou", name="ou", bufs=1)
                                for i in range(4):
                                    nc.tensor.matmul(
                                        ou[:, 32 * i:32 * (i + 1)],
                                        bc(pt[:, 128 * i:128 * (i + 1)]),
                                        bc(vtm[wl][:, 128 * G + 32 * i:
                                                   128 * G + 32 * (i + 1)]),
                                        start=True, stop=True,
                                        skip_group_check=True)
                                on4 = pw.tile([128, 128], F32, tag="on4", name="on4")
                                for i in range(4):
                                    nc.vector.tensor_scalar_mul(
                                        on4[:, 32 * i:32 * (i + 1)],
                                        ou[:, 32 * i:32 * (i + 1)],
                                        rv[:, i:i + 1])
                                nc.tensor.matmul(
                                    otb[:, 128 * wl:128 * (wl + 1)],
                                    on4, ident, is_transpose=True,
                                    start=False, stop=(wl == 3),
                                    skip_group_check=True)

                            for wl in range(4):
                                pt = emit_front(wl)
                                emit_back(wl, pt)
                            # lepe bias + copy out
                            nc.scalar.add(
                                bc(attT[2 * br + G][:, 512 * half:512 * (half + 1)]),
                                otb, lb[br][:, G:G + 1])

                if stage == 3:
                    for ch in range(NCH):
                        nc.sync.dma_start(
                            out=out_d[128 * ch:128 * (ch + 1), 0:TOK], in_=attT[ch])
                    continue
                # ---- proj + residual -> xf scratch ----
                for oc in range(NCH):
                    xfo = pxfo.tile([128, TOK], F32, tag="xfo", name="xfo")
                    for g2 in range(2):
                        pp = ps_mm.tile([128, 512], F32, tag="mm", name="mm")
                        nc.tensor.matmul(pp, bc(pb[0:1, 128 * oc:128 * (oc + 1)]),
                                         bc(ones1), start=True, stop=False)
                        for k in range(NCH):
                            if k < 2:  # branch 0: un-permute window order
                                rhs = attT[k].rearrange(
                                    "p (j h w) -> p h j w", j=8, h=32, w=4
                                )[:, 16 * g2:16 * (g2 + 1), :, :]
                            else:
                                rhs = attT[k][:, 512 * g2:512 * (g2 + 1)]
                            nc.tensor.matmul(
                                pp, bc(projw[k][:, 128 * oc:128 * (oc + 1)]),
                                bc(rhs), start=False, stop=(k == NCH - 1))
                        nc.vector.tensor_add(xfo[:, 512 * g2:512 * (g2 + 1)], pp,
                                             xs[oc][:, 512 * g2:512 * (g2 + 1)])
                    nc.sync.dma_start(
                        out=xf_d[128 * oc:128 * (oc + 1), TOK * sl:TOK * (sl + 1)],
                        in_=xfo)

        # =============== PHASE B (MLP) ===============
        if stage < 5:
            nc.compile()
            return nc
        with ExitStack() as bctx:
            wB = bctx.enter_context(tc.tile_pool(name="wB", bufs=1))
            pxf = bctx.enter_context(tc.tile_pool(name="pxf", bufs=8))
            phn = bctx.enter_context(tc.tile_pool(name="phn", bufs=8))
            ph = bctx.enter_context(tc.tile_pool(name="ph", bufs=NHC))
            psqB = bctx.enter_context(tc.tile_pool(name="psqB", bufs=4))
            pstatB = bctx.enter_context(tc.tile_pool(name="pstatB", bufs=1))
            pout = bctx.enter_context(tc.tile_pool(name="pout", bufs=4))
            psB = bctx.enter_context(tc.tile_pool(name="psB", bufs=4, space="PSUM"))

            fc1w = []
            for k in range(NCH):
                t = wB.tile([128, HID], F32, tag=f"fc1w{k}", name=f"fc1w{k}")
                nc.sync.dma_start(out=bc(t), in_=bc(dram["fc1_w"][128 * k:128 * (k + 1), :]))
                fc1w.append(t)
            fc2w = []
            for k in range(NHC):
                t = wB.tile([128, C], F32, tag=f"fc2w{k}", name=f"fc2w{k}")
                nc.sync.dma_start(out=bc(t), in_=bc(dram["fc2_w"][128 * k:128 * (k + 1), :]))
                fc2w.append(t)

            lb = loops_b if loops_b is not None else loops
            loopB = tc.For_i(0, lb, 1) if lb > 1 else contextlib.nullcontext()
            with loopB:
              for gp in range(TCORE // 1024):
                xfb = []
                for ch in range(NCH):
                    t = pxf.tile([128, 1024], F32, tag="xfb", name="xfb")
                    nc.sync.dma_start(
                        out=bc(t), in_=bc(xf_d[128 * ch:128 * (ch + 1),
                                               1024 * gp:1024 * (gp + 1)]))
                    xfb.append(t)
                ots = [pout.tile([128, 1024], F32, tag="ot", name="ot")
                       for _ in range(NCH)]
                for h2 in range(2):
                    hn = [phn.tile([128, 512], F32, tag="hn", name="hn")
                          for _ in range(NCH)]
                    ln_group(lambda ch: xfb[ch][:, 512 * h2:512 * (h2 + 1)],
                             lambda ch: hn[ch],
                             g2t, b2t, (psqB, pstatB, psB))
                    hs = []
                    for hc in range(NHC):
                        pp = psB.tile([128, 512], F32, tag="mm", name="mm")
                        for k in range(NCH):
                            nc.tensor.matmul(pp, bc(fc1w[k][:, 128 * hc:128 * (hc + 1)]),
                                             bc(hn[k]), start=(k == 0), stop=(k == NCH - 1))
                        t = ph.tile([128, 512], F32, tag="h", name="h")
                        nc.scalar.activation(bc(t), pp, gelu_func, bias=fc1b[:, hc:hc + 1])
                        hs.append(t)
                    for oc in range(NCH):
                        pp = psB.tile([128, 512], F32, tag="mm", name="mm")
                        nc.tensor.matmul(pp, bc(fc2b[0:1, 128 * oc:128 * (oc + 1)]),
                                         bc(ones1), start=True, stop=False)
                        for k in range(NHC):
                            nc.tensor.matmul(pp, bc(fc2w[k][:, 128 * oc:128 * (oc + 1)]),
                                             bc(hs[k]), start=False, stop=(k == NHC - 1))
                        nc.vector.tensor_add(ots[oc][:, 512 * h2:512 * (h2 + 1)],
                                             pp, xfb[oc][:, 512 * h2:512 * (h2 + 1)])
                for oc in range(NCH):
                    nc.sync.dma_start(
                        out=out_d[128 * oc:128 * (oc + 1), 1024 * gp:1024 * (gp + 1)],
                        in_=ots[oc])

    nc.compile()
    return nc


_NC = None


def _get_nc():
    global _NC
    if _NC is None:
        _NC = build_kernel()
    return _NC


def make_in_maps(inputs):
    f = lambda a: np.ascontiguousarray(np.asarray(a), dtype=np.float32)
    x = f(inputs["x"])  # [1, C, 32, 32, 32]
    shared = {
        "norm1_g": f(inputs["norm1_g"]), "norm1_b": f(inputs["norm1_b"]),
        "qkv_w": f(inputs["qkv_w"]),
        "lepe0_w": f(inputs["lepe0_w"]).reshape(CB, 9),
        "lepe0_b": f(inputs["lepe0_b"]),
        "lepe1_w": f(inputs["lepe1_w"]).reshape(CB, 9),
        "lepe1_b": f(inputs["lepe1_b"]),
        "proj_w": f(inputs["proj_w"]), "proj_b": f(inputs["proj_b"]),
        "norm2_g": f(inputs["norm2_g"]), "norm2_b": f(inputs["norm2_b"]),
        "fc1_w": f(inputs["fc1_w"]), "fc1_b": f(inputs["fc1_b"]),
        "fc2_w": f(inputs["fc2_w"]), "fc2_b": f(inputs["fc2_b"]),
    }
    in_maps = []
    for i in range(N_CORES):
        m = dict(shared)
        m["x"] = np.ascontiguousarray(
            x[0, :, NSLICE * i:NSLICE * (i + 1)].reshape(C, TCORE))
        in_maps.append(m)
    return in_maps


def kernel(**inputs):
    from concourse.bass_utils import run_bass_kernel_spmd
    nc = _get_nc()
    in_maps = make_in_maps(inputs)
    res = run_bass_kernel_spmd(nc, in_maps, core_ids=list(range(N_CORES)))
    out = np.empty((1, C, RESO, RESO, RESO), dtype=np.float32)
    for i in range(N_CORES):
        out[0, :, NSLICE * i:NSLICE * (i + 1)] = (
            res.results[i]["out"].reshape(C, NSLICE, RESO, RESO))
    return out



# revision 25
# speedup vs baseline: 12105.4220x; 12105.4220x over previous
"""CSWinBlock3D Trainium2 kernel (8-core SPMD, data-parallel over depth).

Layout: channels-major [C, T] (matches x's DRAM layout [1, C, D, H, W]).
Each core handles 4 depth slices = 4096 tokens. No collectives.

v2: bf16 GEMMs (FWL weight loads), LN gain folded into qkv/fc1 weights,
bias matmuls folded into DVE evacuations, bf16 attention operands,
ACT limited to Ln/Exp/Gelu tables.
"""

import sys

sys.path.insert(0, "/opt/trn_rl_repo")

from contextlib import ExitStack

import numpy as np

import concourse.bass as bass
import concourse.bacc as bacc
import concourse.tile as tile
from concourse import mybir

F32 = mybir.dt.float32
F32R = mybir.dt.float32r
BF16 = mybir.dt.bfloat16
AF = mybir.ActivationFunctionType
ALU = mybir.AluOpType

N_CORES = 8
C = 512
RESO = 32
SPLIT = 4
HH = 8          # heads per branch
HD = 32         # head dim
CB = 256        # channels per branch
HID = 2048
EPS = 1e-5
SCALE = HD ** -0.5
NSLICE = 4      # depth slices per core
TOK = 1024      # tokens per depth slice
TCORE = NSLICE * TOK  # 4096 tokens per core
NCH = C // 128  # 4 channel chunks
NHC = HID // 128  # 16 hidden chunks


def bc(ap):
    return ap.bitcast(F32R)


def build_kernel(gelu_func=AF.Gelu, stage=5, loops=1, loops_a=None, loops_b=None):
    nc = bacc.Bacc("TRN2", target_bir_lowering=False, debug=False,
                   num_devices=N_CORES)

    dram = {}
    def din(name, shape):
        dram[name] = nc.dram_tensor(name, list(shape), F32, kind="ExternalInput").ap()
    din("x", (C, TCORE))
    din("norm1_g", (C,)); din("norm1_b", (C,))
    din("qkv_w", (C, 3 * C))
    din("lepe0_w", (CB, 9)); din("lepe0_b", (CB,))
    din("lepe1_w", (CB, 9)); din("lepe1_b", (CB,))
    din("proj_w", (C, C)); din("proj_b", (C,))
    din("norm2_g", (C,)); din("norm2_b", (C,))
    din("fc1_w", (C, HID)); din("fc1_b", (HID,))
    din("fc2_w", (HID, C)); din("fc2_b", (C,))
    out_d = nc.dram_tensor("out", [C, TCORE], F32, kind="ExternalOutput").ap()
    xf_d = nc.dram_tensor("xf_scratch", [C, TCORE], F32, kind="Internal").ap()

    import ml_dtypes
    ident_d = nc.inline_tensor(
        np.eye(128, dtype=np.float32), name="ident128")
    identb_d = nc.inline_tensor(
        np.eye(128, dtype=ml_dtypes.bfloat16), name="identb128")
    onesb_d = nc.inline_tensor(
        np.ones((128, 128), dtype=ml_dtypes.bfloat16), name="onesb128")
    zeros_d = nc.inline_tensor(
        np.zeros((128, 8 * 204), dtype=ml_dtypes.bfloat16), name="zerosc")

    with ExitStack() as ctx:
        tc = ctx.enter_context(tile.TileContext(nc))
        csts = ctx.enter_context(tc.tile_pool(name="csts", bufs=1))

        # ---- constants ----
        onesb = csts.tile([128, 128], BF16, tag="onesb", name="onesb")
        nc.sync.dma_start(out=onesb, in_=onesb_d.ap())
        eps_t = csts.tile([128, 1], F32, tag="eps_t", name="eps_t")
        nc.gpsimd.memset(eps_t, EPS)
        zero_t = csts.tile([128, 1], F32, tag="zero_t", name="zero_t")
        nc.gpsimd.memset(zero_t, 0.0)

        def load_pcol(name, nchunk):
            # [nchunk*128] dram -> [128, nchunk] sbuf (col c = chunk c)
            t = csts.tile([128, nchunk], F32, tag=name, name=name)
            nc.sync.dma_start(out=t, in_=dram[name].rearrange("(c p) -> p c", p=128))
            return t
        g1t = load_pcol("norm1_g", NCH); b1t = load_pcol("norm1_b", NCH)
        g2t = load_pcol("norm2_g", NCH); b2t = load_pcol("norm2_b", NCH)
        fc1b = load_pcol("fc1_b", NHC)
        pbt = load_pcol("proj_b", NCH)
        fc2bt = load_pcol("fc2_b", NCH)
        b1b = csts.tile([128, NCH], BF16, tag="b1b", name="b1b")
        nc.vector.tensor_copy(b1b, b1t)
        b2b = csts.tile([128, NCH], BF16, tag="b2b", name="b2b")
        nc.vector.tensor_copy(b2b, b2t)

        lb = []
        lw = []
        for br in range(2):
            lwn = f"lepe{br}_w"
            lwt = []
            for ch in range(2):
                t = csts.tile([128, 9], F32, tag=f"{lwn}_{ch}", name=f"{lwn}_{ch}")
                nc.sync.dma_start(out=t, in_=dram[lwn][128 * ch:128 * (ch + 1), :])
                lwt.append(t)
            lw.append(lwt)
            lbn = f"lepe{br}_b"
            t = csts.tile([128, 2], F32, tag=lbn, name=lbn)
            nc.sync.dma_start(out=t, in_=dram[lbn].rearrange("(c p) -> p c", p=128))
            lb.append(t)

        # =============== helpers ===============
        def ln_stats(xb_ap, pools):
            """LayerNorm stats for one 512-token group from bf16 input.

            Returns (negm, rb) as [128, 512] f32 tiles (replicated rows)."""
            psq, pstat, ps = pools
            xsq = []
            for ch in range(NCH):
                t = psq.tile([128, 512], BF16, tag="xsq", name="xsq")
                nc.vector.tensor_mul(t, xb_ap(ch), xb_ap(ch))
                xsq.append(t)
            sb = ps.tile([128, 512], F32, tag="mm", name="mm")
            for k in range(NCH):
                nc.tensor.matmul(sb, onesb, xb_ap(k),
                                 start=(k == 0), stop=(k == NCH - 1))
            qb = ps.tile([128, 512], F32, tag="mm", name="mm")
            for k in range(NCH):
                nc.tensor.matmul(qb, onesb, xsq[k],
                                 start=(k == 0), stop=(k == NCH - 1))
            negm = pstat.tile([128, 512], F32, tag="negm", name="negm", bufs=2)
            nc.vector.tensor_scalar_mul(negm, sb, -1.0 / C)
            tq = pstat.tile([128, 512], F32, tag="tq", name="tq")
            nc.vector.tensor_scalar_mul(tq, qb, 1.0 / C)
            m2 = pstat.tile([128, 512], F32, tag="m2", name="m2")
            nc.vector.tensor_mul(m2, negm, negm)
            var = pstat.tile([128, 512], F32, tag="var", name="var")
            nc.vector.tensor_sub(var, tq, m2)
            sd = pstat.tile([128, 512], F32, tag="sd", name="sd")
            nc.scalar.activation(sd, var, AF.Ln, bias=eps_t)
            rb = pstat.tile([128, 512], BF16, tag="rb", name="rb", bufs=2)
            nc.scalar.activation(rb, sd, AF.Exp, bias=zero_t, scale=-0.5)
            return negm, rb

        def ln_apply2(xb_ap, dst_ap, negm, rb, pstat):
            # dst = (xb + negm) * rb   (gain/bias folded into the weights)
            for ch in range(NCH):
                u = pstat.tile([128, 512], F32, tag="u", name="u")
                nc.vector.tensor_add(u, xb_ap(ch), negm)
                nc.vector.tensor_mul(dst_ap(ch), u, rb)

        def ln_group(xb_ap, dst_ap, pools):
            negm, rb = ln_stats(xb_ap, pools)
            ln_apply2(xb_ap, dst_ap, negm, rb, pools[1])

        # =============== PHASE A ===============
        with ExitStack() as actx:
            wA = actx.enter_context(tc.tile_pool(name="wA", bufs=1))
            wstg = actx.enter_context(tc.tile_pool(name="wstg", bufs=2))
            px = actx.enter_context(tc.tile_pool(name="px", bufs=6))
            pxb = actx.enter_context(tc.tile_pool(name="pxb", bufs=8))
            ident = wA.tile([128, 128], F32, tag="ident", name="ident")
            nc.sync.dma_start(out=ident, in_=ident_d.ap())
            identb = wA.tile([128, 128], BF16, tag="identb", name="identb")
            nc.sync.dma_start(out=identb, in_=identb_d.ap())

            # prefetch slice 0's x ahead of the weight loads/casts so the
            # LN1 stats matmuls can start while weights stream in
            xs0, xb0 = [], []
            for ch in range(NCH):
                t = px.tile([128, TOK], F32, tag="x", name="x")
                nc.sync.dma_start(out=bc(t), in_=bc(dram["x"][128 * ch:128 * (ch + 1), 0:TOK]))
                xs0.append(t)
                tb_ = pxb.tile([128, TOK], BF16, tag="xb", name="xb")
                nc.vector.tensor_copy(tb_, t)
                xb0.append(tb_)
            # diag matrices for lepe: dgb[br][ch][tap] = diag(w[128ch.., tap])
            dgb = [[[None] * 9 for _ in range(2)] for _ in range(2)]
            for br in range(2):
                for ch in range(2):
                    for tap in range(9):
                        t = wA.tile([128, 128], BF16, tag=f"dgb{br}{ch}{tap}",
                                    name=f"dgb{br}{ch}{tap}")
                        nc.vector.tensor_scalar_mul(t, ident,
                                                    lw[br][ch][:, tap:tap + 1])
                        dgb[br][ch][tap] = t
            # qkv weights: bf16, LN1 gain folded in (row c scaled by g1[c])
            qkvw = []
            for k in range(NCH):
                stg = wstg.tile([128, 3 * C], F32, tag="stgq", name="stgq")
                nc.sync.dma_start(
                    out=bc(stg), in_=bc(dram["qkv_w"][128 * k:128 * (k + 1), :]))
                t = wA.tile([128, 3 * C], BF16, tag=f"qkvw{k}", name=f"qkvw{k}")
                nc.vector.tensor_scalar_mul(t, stg, g1t[:, k:k + 1])
                qkvw.append(t)
            projw = []
            for k in range(NCH):
                stg = wstg.tile([128, C], F32, tag="stgp", name="stgp")
                nc.sync.dma_start(
                    out=bc(stg), in_=bc(dram["proj_w"][128 * k:128 * (k + 1), :]))
                t = wA.tile([128, C], BF16, tag=f"projw{k}", name=f"projw{k}")
                nc.vector.tensor_copy(t, stg)
                projw.append(t)

            pimg = actx.enter_context(tc.tile_pool(name="pimg", bufs=6))
            pattT = actx.enter_context(tc.tile_pool(name="pattT", bufs=6))
            pqkv = actx.enter_context(tc.tile_pool(name="pqkv", bufs=1))
            psq = actx.enter_context(tc.tile_pool(name="psq", bufs=4))
            pstat = actx.enter_context(tc.tile_pool(name="pstat", bufs=1))
            pw = actx.enter_context(tc.tile_pool(name="pw", bufs=4))
            pvtm = actx.enter_context(tc.tile_pool(name="pvtm", bufs=4))
            pxfo = actx.enter_context(tc.tile_pool(name="pxfo", bufs=2))
            pvpad = actx.enter_context(tc.tile_pool(name="pvpad", bufs=1))
            # zero-halo V buffers: per (branch, chunk), halo zeroed once
            vpad = [[pvpad.tile([128, 8 * 204], BF16, tag=f"vpad{b}{ch}",
                                name=f"vpad{b}{ch}") for ch in range(2)]
                    for b in range(2)]
            for b in range(2):
                for ch in range(2):
                    nc.sync.dma_start(out=vpad[b][ch], in_=zeros_d.ap())
            ps_mm = actx.enter_context(tc.tile_pool(name="ps_mm", bufs=2, space="PSUM"))
            ps_ot = actx.enter_context(tc.tile_pool(name="ps_ot", bufs=2, space="PSUM"))
            ps_sm = actx.enter_context(tc.tile_pool(name="ps_sm", bufs=2, space="PSUM"))

            # qkv bias correction: qbias = b1 @ (g1*qkv_w)  -> [128, 12]
            qb_ps = ps_mm.tile([128, 12], F32, tag="mm", name="qbias_ps")
            for oc in range(12):
                for k in range(NCH):
                    nc.tensor.matmul(qb_ps[:, oc:oc + 1],
                                     qkvw[k][:, 128 * oc:128 * (oc + 1)],
                                     b1b[:, k:k + 1],
                                     start=(k == 0), stop=(k == NCH - 1))
            qbias = wA.tile([128, 12], F32, tag="qbias", name="qbias")
            nc.vector.tensor_copy(qbias, qb_ps)

            import contextlib
            la = loops_a if loops_a is not None else loops
            loopA = tc.For_i(0, la, 1) if la > 1 else contextlib.nullcontext()
            with loopA:
              for sl in range(NSLICE):
                # load x slice (channels-major, raw token order)
                if sl == 0:
                    xs, xb = xs0, xb0
                else:
                    xs = []
                    xb = []
                    for ch in range(NCH):
                        t = px.tile([128, TOK], F32, tag="x", name="x")
                        nc.sync.dma_start(
                            out=bc(t), in_=bc(dram["x"][128 * ch:128 * (ch + 1),
                                                        TOK * sl:TOK * (sl + 1)]))
                        xs.append(t)
                        tb_ = pxb.tile([128, TOK], BF16, tag="xb", name="xb")
                        nc.vector.tensor_copy(tb_, t)
                        xb.append(tb_)

                # LN1 -> img (bf16; gain folded into qkv weights)
                img = [pimg.tile([128, TOK], BF16, tag="img", name="img")
                       for _ in range(NCH)]
                for g2 in range(2):
                    ln_group(lambda ch: xb[ch][:, 512 * g2:512 * (g2 + 1)],
                             lambda ch: img[ch][:, 512 * g2:512 * (g2 + 1)],
                             (psq, pstat, ps_mm))

                attT = [pattT.tile([128, TOK], BF16, tag="attT", name="attT")
                        for _ in range(NCH)]

                for br in range(2):
                    # ---- qkv for this branch (window-ordered for br 0) ----
                    # q,k: head-folded [32, 4 heads x 1024 tok] bf16
                    qkf = {}
                    vb = []
                    for m in range(3):  # q, k, v
                        for G in range(2):
                            oc = 4 * m + 2 * br + G
                            if m < 2:
                                tb = pqkv.tile([128, TOK], BF16, tag=f"qkb{m}{G}",
                                               name=f"qkb{m}{G}")
                                t = pqkv.tile([32, 4 * TOK], BF16,
                                              tag=f"qkf{m}{G}", name=f"qkf{m}{G}")
                            else:
                                t = pqkv.tile([128, TOK], BF16, tag=f"qkv{m}{G}",
                                              name=f"qkv{m}{G}")
                            for g2 in range(2):
                                pp = ps_mm.tile([128, 512], F32, tag="mm", name="mm")
                                for k in range(NCH):
                                    if br == 0:
                                        rhs = img[k].rearrange(
                                            "p (h j w) -> p j h w", h=32, j=8, w=4
                                        )[:, 4 * g2:4 * (g2 + 1), :, :]
                                    else:
                                        rhs = img[k][:, 512 * g2:512 * (g2 + 1)]
                                    nc.tensor.matmul(
                                        pp, qkvw[k][:, 128 * oc:128 * (oc + 1)],
                                        rhs, start=(k == 0), stop=(k == NCH - 1))
                                dst = (tb if m < 2 else t)
                                nc.vector.tensor_scalar_add(
                                    dst[:, 512 * g2:512 * (g2 + 1)], pp,
                                    qbias[:, oc:oc + 1])
                            if m < 2:
                                for i in range(4):
                                    nc.sync.dma_start(
                                        out=t[0:32, 1024 * i:1024 * (i + 1)],
                                        in_=tb[32 * i:32 * (i + 1), :])
                                qkf[(m, G)] = t
                            else:
                                vb.append(t)
                    qf = [qkf[(0, 0)], qkf[(0, 1)]]
                    kf = [qkf[(1, 0)], qkf[(1, 1)]]

                    # ---- attention ----
                    Y, X = (32, 4) if br == 0 else (4, 32)
                    # fill zero-halo V interiors for lepe (one strided copy per chunk)
                    for ch2 in range(2):
                        nc.vector.tensor_copy(
                            vpad[br][ch2].rearrange(
                                "p (s y x) -> p s y x", s=8, y=Y + 2, x=X + 2
                            )[:, :, 1:Y + 1, 1:X + 1],
                            vb[ch2].rearrange(
                                "p (s y x) -> p s y x", s=8, y=Y, x=X))
                    for half in range(2):
                        # V tokens-major for the 4 windows of this half (bf16)
                        vtm = []
                        for wl in range(4):
                            win = 4 * half + wl
                            tp = ps_sm.tile([128, 256], BF16, tag="sm", name="smv")
                            for ch2 in range(2):
                                nc.tensor.transpose(
                                    tp[:, 128 * ch2:128 * (ch2 + 1)],
                                    vb[ch2][:, 128 * win:128 * (win + 1)],
                                    identb)
                            vt = pvtm.tile([128, 256], BF16, tag="vtm", name="vtm")
                            nc.vector.tensor_copy(vt, tp)
                            vtm.append(vt)
                        for G in range(2):
                            otb = ps_ot.tile([128, 512], F32, tag="ot", name="ot")
                            # lepe depthwise taps (center first: start=True)
                            taps = [(1, 1)] + [(dy, dx) for dy in range(3)
                                               for dx in range(3) if (dy, dx) != (1, 1)]
                            for (dy, dx) in taps:
                                srcap = vpad[br][G].rearrange(
                                    "p (s y x) -> p s y x", s=8, y=Y + 2, x=X + 2
                                )[:, 4 * half:4 * (half + 1),
                                  dy:dy + Y, dx:dx + X]
                                nc.tensor.matmul(
                                    otb, dgb[br][G][3 * dy + dx],
                                    srcap, start=(dy == 1 and dx == 1),
                                    stop=False, skip_group_check=True)
                            def emit_front(wl):
                                win = 4 * half + wl
                                sx = ps_sm.tile([128, 512], F32, tag="sm", name="sm")
                                for i in range(4):
                                    nc.tensor.matmul(
                                        sx[:, 128 * i:128 * (i + 1)],
                                        kf[G][0:32, 1024 * i + 128 * win:
                                              1024 * i + 128 * (win + 1)],
                                        qf[G][0:32, 1024 * i + 128 * win:
                                              1024 * i + 128 * (win + 1)],
                                        start=True, stop=True,
                                        skip_group_check=True)
                                pt = pw.tile([128, 512], BF16, tag="pt", name="pt")
                                nc.scalar.activation(pt, sx, AF.Exp, bias=zero_t,
                                                     scale=SCALE)
                                return pt

                            def emit_back(wl, pt):
                                # one bank: cols 0-127 = attn@V, 128-135 = row sums
                                ousv = ps_sm.tile([128, 136], F32, tag="ousv",
                                                  name="ousv", bufs=2)
                                ou = ousv[:, 0:128]
                                sv = ousv[:, 128:136]
                                for i in range(4):
                                    nc.tensor.matmul(
                                        sv[:, 2 * i:2 * i + 2],
                                        pt[:, 128 * i:128 * (i + 1)],
                                        onesb[:, 0:2],
                                        start=True, stop=True,
                                        skip_group_check=True)
                                rv = pw.tile([128, 4], F32, tag="rv", name="rv")
                                nc.vector.reciprocal(rv, sv.rearrange(
                                    "p (a b) -> p a b", a=4, b=2)[:, :, 0])
                                for i in range(4):
                                    nc.tensor.matmul(
                                        ou[:, 32 * i:32 * (i + 1)],
                                        pt[:, 128 * i:128 * (i + 1)],
                                        vtm[wl][:, 128 * G + 32 * i:
                                                128 * G + 32 * (i + 1)],
                                        start=True, stop=True,
                                        skip_group_check=True)
                                on4 = pw.tile([128, 128], F32, tag="on4", name="on4")
                                nc.vector.tensor_mul(
                                    on4.rearrange("p (a b) -> p a b", a=4, b=32),
                                    ou.rearrange("p (a b) -> p a b", a=4, b=32),
                                    rv.rearrange("p (a b) -> p a b", a=4, b=1
                                                 ).broadcast_to([128, 4, 32]))
                                nc.tensor.matmul(
                                    otb[:, 128 * wl:128 * (wl + 1)],
                                    on4, ident, is_transpose=True,
                                    start=False, stop=(wl == 3),
                                    skip_group_check=True)

                            for wl in range(4):
                                pt = emit_front(wl)
                                emit_back(wl, pt)
                            # lepe bias + copy out (DVE)
                            nc.vector.tensor_scalar_add(
                                attT[2 * br + G][:, 512 * half:512 * (half + 1)],
                                otb, lb[br][:, G:G + 1])

                # ---- proj + residual -> xf scratch ----
                for oc in range(NCH):
                    xfo = pxfo.tile([128, TOK], F32, tag="xfo", name="xfo")
                    for g2 in range(2):
                        pp = ps_mm.tile([128, 512], F32, tag="mm", name="mm")
                        for k in range(NCH):
                            if k < 2:  # branch 0: un-permute window order
                                rhs = attT[k].rearrange(
                                    "p (j h w) -> p h j w", j=8, h=32, w=4
                                )[:, 16 * g2:16 * (g2 + 1), :, :]
                            else:
                                rhs = attT[k][:, 512 * g2:512 * (g2 + 1)]
                            nc.tensor.matmul(
                                pp, projw[k][:, 128 * oc:128 * (oc + 1)],
                                rhs, start=(k == 0), stop=(k == NCH - 1))
                        # xfo = (pp + proj_b) + x
                        nc.vector.scalar_tensor_tensor(
                            xfo[:, 512 * g2:512 * (g2 + 1)], pp,
                            pbt[:, oc:oc + 1],
                            xs[oc][:, 512 * g2:512 * (g2 + 1)],
                            op0=ALU.add, op1=ALU.add)
                    nc.sync.dma_start(
                        out=xf_d[128 * oc:128 * (oc + 1), TOK * sl:TOK * (sl + 1)],
                        in_=xfo)

        # =============== PHASE B (MLP) ===============
        with ExitStack() as bctx:
            wB = bctx.enter_context(tc.tile_pool(name="wB", bufs=1))
            wstgB = bctx.enter_context(tc.tile_pool(name="wstgB", bufs=2))
            pxf = bctx.enter_context(tc.tile_pool(name="pxf", bufs=8))
            pxb2 = bctx.enter_context(tc.tile_pool(name="pxb2", bufs=8))
            phn = bctx.enter_context(tc.tile_pool(name="phn", bufs=8))
            ph = bctx.enter_context(tc.tile_pool(name="ph", bufs=NHC))
            psqB = bctx.enter_context(tc.tile_pool(name="psqB", bufs=4))
            pstatB = bctx.enter_context(tc.tile_pool(name="pstatB", bufs=1))
            pout = bctx.enter_context(tc.tile_pool(name="pout", bufs=4))
            psB = bctx.enter_context(tc.tile_pool(name="psB", bufs=4, space="PSUM"))

            # prefetch gp 0's xf ahead of the weight loads/casts so LN2
            # stats can start immediately after phase A's last xf write
            xfb0, xb20 = [], []
            for ch in range(NCH):
                t = pxf.tile([128, 1024], F32, tag="xfb", name="xfb")
                nc.sync.dma_start(out=bc(t), in_=bc(xf_d[128 * ch:128 * (ch + 1), 0:1024]))
                xfb0.append(t)
                tb_ = pxb2.tile([128, 1024], BF16, tag="xb2", name="xb2")
                nc.vector.tensor_copy(tb_, t)
                xb20.append(tb_)

            fc1w = []
            for k in range(NCH):
                stg = wstgB.tile([128, HID], F32, tag="stg1", name="stg1")
                nc.sync.dma_start(
                    out=bc(stg), in_=bc(dram["fc1_w"][128 * k:128 * (k + 1), :]))
                t = wB.tile([128, HID], BF16, tag=f"fc1w{k}", name=f"fc1w{k}")
                nc.vector.tensor_scalar_mul(t, stg, g2t[:, k:k + 1])
                fc1w.append(t)
            fc2w = []
            for k in range(NHC):
                stg = wstgB.tile([128, C], F32, tag="stg2", name="stg2")
                nc.sync.dma_start(
                    out=bc(stg), in_=bc(dram["fc2_w"][128 * k:128 * (k + 1), :]))
                t = wB.tile([128, C], BF16, tag=f"fc2w{k}", name=f"fc2w{k}")
                nc.vector.tensor_copy(t, stg)
                fc2w.append(t)

            # fc1 bias correction: fc1b_eff = fc1_b + b2 @ (g2*fc1_w)
            fb_ps = psB.tile([128, NHC], F32, tag="mm", name="fbias_ps")
            for hc in range(NHC):
                for k in range(NCH):
                    nc.tensor.matmul(fb_ps[:, hc:hc + 1],
                                     fc1w[k][:, 128 * hc:128 * (hc + 1)],
                                     b2b[:, k:k + 1],
                                     start=(k == 0), stop=(k == NCH - 1))
            fc1be = wB.tile([128, NHC], F32, tag="fc1be", name="fc1be")
            nc.vector.tensor_add(fc1be, fb_ps, fc1b)

            import contextlib
            lbv = loops_b if loops_b is not None else loops
            loopB = tc.For_i(0, lbv, 1) if lbv > 1 else contextlib.nullcontext()
            with loopB:
              for gp in range(TCORE // 1024):
                if gp == 0:
                    xfb, xb2 = xfb0, xb20
                else:
                    xfb = []
                    xb2 = []
                    for ch in range(NCH):
                        t = pxf.tile([128, 1024], F32, tag="xfb", name="xfb")
                        nc.sync.dma_start(
                            out=bc(t), in_=bc(xf_d[128 * ch:128 * (ch + 1),
                                                   1024 * gp:1024 * (gp + 1)]))
                        xfb.append(t)
                        tb_ = pxb2.tile([128, 1024], BF16, tag="xb2", name="xb2")
                        nc.vector.tensor_copy(tb_, t)
                        xb2.append(tb_)
                ots = [pout.tile([128, 1024], F32, tag="ot", name="ot")
                       for _ in range(NCH)]
                # both halves' LN stats first (keeps exp/ln table loaded),
                # then the applies + GEMMs + gelu
                stats = []
                for h2 in range(2):
                    stats.append(ln_stats(
                        lambda ch: xb2[ch][:, 512 * h2:512 * (h2 + 1)],
                        (psqB, pstatB, psB)))
                for h2 in range(2):
                    negm, rb = stats[h2]
                    hn = [phn.tile([128, 512], BF16, tag="hn", name="hn")
                          for _ in range(NCH)]
                    ln_apply2(lambda ch: xb2[ch][:, 512 * h2:512 * (h2 + 1)],
                              lambda ch: hn[ch], negm, rb, pstatB)
                    hs = []
                    for hc in range(NHC):
                        pp = psB.tile([128, 512], F32, tag="mm", name="mm")
                        for k in range(NCH):
                            nc.tensor.matmul(pp, fc1w[k][:, 128 * hc:128 * (hc + 1)],
                                             hn[k], start=(k == 0), stop=(k == NCH - 1))
                        t = ph.tile([128, 512], BF16, tag="h", name="h")
                        nc.scalar.activation(t, pp, gelu_func,
                                             bias=fc1be[:, hc:hc + 1])
                        hs.append(t)
                    for oc in range(NCH):
                        pp = psB.tile([128, 512], F32, tag="mm", name="mm")
                        for k in range(NHC):
                            nc.tensor.matmul(pp, fc2w[k][:, 128 * oc:128 * (oc + 1)],
                                             hs[k], start=(k == 0), stop=(k == NHC - 1))
                        # ots = (pp + fc2_b) + xf
                        nc.vector.scalar_tensor_tensor(
                            ots[oc][:, 512 * h2:512 * (h2 + 1)], pp,
                            fc2bt[:, oc:oc + 1],
                            xfb[oc][:, 512 * h2:512 * (h2 + 1)],
                            op0=ALU.add, op1=ALU.add)
                for oc in range(NCH):
                    nc.sync.dma_start(
                        out=out_d[128 * oc:128 * (oc + 1), 1024 * gp:1024 * (gp + 1)],
                        in_=ots[oc])

    nc.compile()
    return nc


_NC = None


def _get_nc():
    global _NC
    if _NC is None:
        _NC = build_kernel()
    return _NC


def make_in_maps(inputs):
    f = lambda a: np.ascontiguousarray(np.asarray(a), dtype=np.float32)
    x = f(inputs["x"])  # [1, C, 32, 32, 32]
    shared = {
        "norm1_g": f(inputs["norm1_g"]), "norm1_b": f(inputs["norm1_b"]),
        "qkv_w": f(inputs["qkv_w"]),
        "lepe0_w": f(inputs["lepe0_w"]).reshape(CB, 9),
        "lepe0_b": f(inputs["lepe0_b"]),
        "lepe1_w": f(inputs["lepe1_w"]).reshape(CB, 9),
        "lepe1_b": f(inputs["lepe1_b"]),
        "proj_w": f(inputs["proj_w"]), "proj_b": f(inputs["proj_b"]),
        "norm2_g": f(inputs["norm2_g"]), "norm2_b": f(inputs["norm2_b"]),
        "fc1_w": f(inputs["fc1_w"]), "fc1_b": f(inputs["fc1_b"]),
        "fc2_w": f(inputs["fc2_w"]), "fc2_b": f(inputs["fc2_b"]),
    }
    in_maps = []
    for i in range(N_CORES):
        m = dict(shared)
        m["x"] = np.ascontiguousarray(
            x[0, :, NSLICE * i:NSLICE * (i + 1)].reshape(C, TCORE))
        in_maps.append(m)
    return in_maps


def kernel(**inputs):
    from concourse.bass_utils import run_bass_kernel_spmd
    nc = _get_nc()
    in_maps = make_in_maps(inputs)
    res = run_bass_kernel_spmd(nc, in_maps, core_ids=list(range(N_CORES)))
    out = np.empty((1, C, RESO, RESO, RESO), dtype=np.float32)
    for i in range(N_CORES):
        out[0, :, NSLICE * i:NSLICE * (i + 1)] = (
            res.results[i]["out"].reshape(C, NSLICE, RESO, RESO))
    return out
